# revision 14
# baseline (speedup 1.0000x reference)
"""Trainium2 Bass kernel for CascadedLoRALinear4bit.

Computes out[b,s,o] = x @ W_base^T + b_base + scaling * (x @ A^T) @ B^T
with scaling == rank/alpha == 1.0.

Strategy:
  - Algebraic fold (exact): out = x @ (W_base + B @ A)^T + b_base.
    The fold is computed on host in fp32 (0.5 GFLOP, negligible).
  - Data-parallel over tokens: the 4*4096 = 16384 tokens are sharded
    8 ways (2048 tokens per NeuronCore). W_eff^T and bias are
    replicated to all cores. No collectives needed.
  - Mixed-precision contraction split: of the 32 k-tiles (128 each),
    the first N8 are computed in fp8 e4m3 with perf_mode=DoubleRow
    (2 k-tiles per matmul, ~2x PE rate), the remaining 32-N8 in bf16.
    N8 is chosen so the worst-case relative error stays ~1.5e-2
    (fp8-only would be 3e-2; bf16-only is 1.9e-3).
  - fp8 scaling to dodge e4m3 subnormals (tiny=2^-6): x is quantized
    as e4m3(16*x), W as e4m3(8*W) -> psum accumulates 128*(x@W^T).
    The bf16 part uses bf16(x) @ bf16(128*W) so the whole PSUM is
    uniformly 128*out. Eviction computes (psum + 128*bias) * (1/128)
    in one DVE tensor_scalar op.
  - Per core: out_c^T[4096, 2048] = W_eff @ x_c^T + bias, tiled for
    the PE with fp32 PSUM accumulation; x_c^T stays fully resident in
    SBUF; W streams through as the stationary operand; each stationary
    tile is reused for 4 moving x chunks of 512 tokens.
  - Output is computed transposed (o on partitions) so the bias is a
    per-partition scalar in the DVE eviction.

Layouts (d = contraction dim on partitions everywhere):
  xT8 [128, 4, N8, 512]     e4m3(16*x), k-tiles 0..N8
  xTb [128, 4, 32-N8, 512]  bf16(x),    k-tiles N8..32
  wT8 [128, 32, N8, 128]    e4m3(8*W),  k-tiles 0..N8
  wTb [128, 32, 32-N8, 128] bf16(128*W)
  bias[128, 32]             128*b_base  (f32)
  out [128, 32, 4, 512]     out[p,nO,mi,s] = out_c[mi*512+s, nO*128+p] (f32)
"""

import sys

if "/opt/trn_rl_repo" not in sys.path:
    sys.path.insert(0, "/opt/trn_rl_repo")

import numpy as np
import ml_dtypes

import concourse.bass as bass
import concourse.mybir as mybir
import concourse.tile as tile
from concourse import bacc
from concourse.bass_utils import run_bass_kernel_spmd

# Problem dims (hardcoded per contract)
BATCH, SEQ, D_IN, D_OUT = 4, 4096, 4096, 4096
SCALING = 1.0  # rank / alpha = 16 / 16

N_CORES = 8
P = 128
S_PER_CORE = BATCH * SEQ // N_CORES  # 2048
KO = D_IN // P                       # 32 contraction tiles
S_TILE = 512
MI = S_PER_CORE // S_TILE            # 4 moving (token) chunks
NO = D_OUT // P                      # 32 output-row blocks

N8 = 10           # k-tiles computed in fp8 DoubleRow (must be even)
NBF = KO - N8     # k-tiles computed in bf16
XS = 16.0         # fp8 x pre-scale
WS = 8.0          # fp8 W pre-scale  (total PSUM scale = XS*WS = 128)
PSUM_SCALE = XS * WS

BF16 = mybir.dt.bfloat16
F8 = mybir.dt.float8e4
F32 = mybir.dt.float32

_compiled = {}


def _build_program(mi_n=MI, no_n=NO, n8=N8, nbf=NBF, s_tile=S_TILE):
    nc = bacc.Bacc(None, target_bir_lowering=False)

    xT8 = nc.declare_dram_parameter("xT8", [P, mi_n, n8, s_tile], F8, isOutput=False)
    xTb = nc.declare_dram_parameter("xTb", [P, mi_n, nbf, s_tile], BF16, isOutput=False)
    wT8 = nc.declare_dram_parameter("wT8", [P, no_n, n8, P], F8, isOutput=False)
    wTb = nc.declare_dram_parameter("wTb", [P, no_n, nbf, P], BF16, isOutput=False)
    bias_d = nc.declare_dram_parameter("bias", [P, no_n], F32, isOutput=False)
    out_d = nc.declare_dram_parameter("out", [P, no_n, mi_n, s_tile], F32, isOutput=True)

    inv_scale = 1.0 / PSUM_SCALE

    with tile.TileContext(nc) as tc:
        with (
            tc.tile_pool(name="xres", bufs=1) as x_pool,
            tc.tile_pool(name="wt", bufs=3) as wt_pool,
            tc.tile_pool(name="bias", bufs=1) as bias_pool,
            tc.tile_pool(name="o", bufs=8) as out_pool,
            tc.tile_pool(name="psum", bufs=2, space="PSUM") as psum_pool,
        ):
            # ---- Startup-latency-aware preload + a paired prologue ----
            # The DMA ring is one FIFO stream (~350 GB/s), so the 13.5 MiB
            # x preload takes ~40us while a single n-block only computes
            # for ~23us.  Blocks 0 and 1 are therefore emitted as a PAIR:
            # both fp8 DoubleRow phases first (their x is only 2.5 MiB),
            # then the two bf16 phases interleaved per k-tile, so the PE
            # consumes each arriving xb chunk twice and stays paced with
            # the stream.  DMA issue order mirrors consumption order.
            xres8 = [x_pool.tile([P, n8, s_tile], F8, name=f"x8_{mi}")
                     for mi in range(mi_n)]
            xresb = [x_pool.tile([P, nbf, s_tile], BF16, name=f"xb_{mi}")
                     for mi in range(mi_n)]

            # PE warm-up: the first real matmul can only start once its
            # DMA lands (~10us in), and the PE then crawls at the low
            # DVFS p-state for the first ~3us of activity.  Run dummy
            # matmuls on a zeroed SBUF tile during the DMA wait so the
            # clock is already ramped; they write the real PSUM tiles,
            # whose first real matmul uses start=True and so discards
            # the garbage.
            scr = x_pool.tile([P, 640], BF16, name="warmup")
            nc.vector.memset(scr[:], 0.0)

            wt80 = wt_pool.tile([P, n8, P], F8, name="wt8")
            nc.sync.dma_start(out=wt80[:], in_=wT8[:, 0, :, :])
            wt81 = wt_pool.tile([P, n8, P], F8, name="wt8")
            nc.sync.dma_start(out=wt81[:], in_=wT8[:, 1, :, :])
            for kc in range(0, n8, 2):
                for mi in range(mi_n):
                    nc.sync.dma_start(
                        out=xres8[mi][:, kc:kc + 2, :],
                        in_=xT8[:, mi, kc:kc + 2, :],
                    )
            wtb0 = wt_pool.tile([P, nbf, P], BF16, name="wtb")
            wtb1 = wt_pool.tile([P, nbf, P], BF16, name="wtb")
            # Stream the bf16 weight k-slices interleaved with the bf16 x
            # chunks in exact consumption order, so the paired bf16 phases
            # below never outrun the FIFO DMA stream.
            nc.sync.dma_start(out=wtb0[:, 0:2, :], in_=wTb[:, 0, 0:2, :])
            nc.sync.dma_start(out=wtb1[:, 0:2, :], in_=wTb[:, 1, 0:2, :])
            for kc in range(0, nbf, 2):
                for mi in range(mi_n):
                    nc.sync.dma_start(
                        out=xresb[mi][:, kc:kc + 2, :],
                        in_=xTb[:, mi, kc:kc + 2, :],
                    )
                nk = kc + 2
                if nk < nbf:
                    ek = min(nk + 2, nbf)
                    nc.sync.dma_start(out=wtb0[:, nk:ek, :],
                                      in_=wTb[:, 0, nk:ek, :])
                    nc.sync.dma_start(out=wtb1[:, nk:ek, :],
                                      in_=wTb[:, 1, nk:ek, :])
            bias_t = bias_pool.tile([P, no_n], F32)
            nc.sync.dma_start(out=bias_t[:], in_=bias_d[:])
            wt82 = wt_pool.tile([P, n8, P], F8, name="wt8")
            nc.sync.dma_start(out=wt82[:], in_=wT8[:, 2, :, :])
            wtb2 = wt_pool.tile([P, nbf, P], BF16, name="wtb")
            nc.sync.dma_start(out=wtb2[:], in_=wTb[:, 2, :, :])
            wt_blks = {0: (wt80, wtb0), 1: (wt81, wtb1), 2: (wt82, wtb2)}

            def dr_phase(pss, wt8_blk, first, last):
                # fp8 DoubleRow pairs: 2 k-tiles per matmul
                for j in range(0, n8, 2):
                    for mi in range(mi_n):
                        nc.tensor.matmul(
                            pss[mi][:],
                            lhsT=wt8_blk[:, j:j + 2, :],
                            rhs=xres8[mi][:, j:j + 2, :],
                            start=(first and j == 0),
                            stop=(last and j == n8 - 2),
                            perf_mode=mybir.MatmulPerfMode.DoubleRow,
                        )

            def evict(pss, n):
                for mi in range(mi_n):
                    ot = out_pool.tile([P, s_tile], F32)
                    # out = (psum + 128*bias) * (1/128)
                    nc.vector.tensor_scalar(
                        ot[:], pss[mi][:],
                        bias_t[:, n:n + 1], inv_scale,
                        mybir.AluOpType.add, mybir.AluOpType.mult,
                    )
                    nc.sync.dma_start(out=out_d[:, n, mi, :], in_=ot[:])

            # Prologue pair: blocks 0 and 1 (uses all 8 PSUM banks)
            pss0 = [psum_pool.tile([P, s_tile], F32, name=f"ps{mi}")
                    for mi in range(mi_n)]
            pss1 = [psum_pool.tile([P, s_tile], F32, name=f"ps{mi}")
                    for mi in range(mi_n)]
            for w in range(24):
                nc.tensor.matmul(
                    pss0[w % mi_n][:],
                    lhsT=scr[:, 0:P],
                    rhs=scr[:, P:P + s_tile],
                    start=True,
                    stop=True,
                )
            # DR phases of blocks 0/1 interleaved per k-pair: each x8
            # chunk is consumed twice back-to-back, pacing the stream.
            for j in range(0, n8, 2):
                for pss, wt8_blk in ((pss0, wt80), (pss1, wt81)):
                    for mi in range(mi_n):
                        nc.tensor.matmul(
                            pss[mi][:],
                            lhsT=wt8_blk[:, j:j + 2, :],
                            rhs=xres8[mi][:, j:j + 2, :],
                            start=(j == 0),
                            stop=False,
                            perf_mode=mybir.MatmulPerfMode.DoubleRow,
                        )
            for k in range(nbf):
                for pss, wtb_blk in ((pss0, wtb0), (pss1, wtb1)):
                    for mi in range(mi_n):
                        nc.tensor.matmul(
                            pss[mi][:],
                            lhsT=wtb_blk[:, k, :],
                            rhs=xresb[mi][:, k, :],
                            start=False,
                            stop=(k == nbf - 1),
                        )
            evict(pss0, 0)
            evict(pss1, 1)

            def bf_phase(pss, wtb_blk, first, last):
                # bf16 k-tiles
                for k in range(nbf):
                    for mi in range(mi_n):
                        nc.tensor.matmul(
                            pss[mi][:],
                            lhsT=wtb_blk[:, k, :],
                            rhs=xresb[mi][:, k, :],
                            start=(first and k == 0),
                            stop=(last and k == nbf - 1),
                        )

            # Steady state: blocks in PAIRS with the phase pattern
            # alternating per pair ([bf,bf,DR,DR] then [DR,DR,bf,bf]),
            # so same-mode matmuls chain across every boundary: one
            # DoubleRow<->normal mode transition per pair instead of
            # two per block.  Each pair uses all 8 PSUM banks (two
            # 4-bank generations of the bufs=2 pool), and a block's
            # banks are evicted one phase before the next pair needs
            # them, so no pipeline bubble.
            for q, na in enumerate(range(2, no_n, 2)):
                nb = na + 1
                blks = []
                for n in (na, nb):
                    if n in wt_blks:
                        blks.append(wt_blks.pop(n))
                    else:
                        w8t = wt_pool.tile([P, n8, P], F8, name="wt8")
                        nc.sync.dma_start(out=w8t[:], in_=wT8[:, n, :, :])
                        wbt = wt_pool.tile([P, nbf, P], BF16, name="wtb")
                        nc.sync.dma_start(out=wbt[:], in_=wTb[:, n, :, :])
                        blks.append((w8t, wbt))
                (wt8_a, wtb_a), (wt8_b, wtb_b) = blks
                pss_a = [psum_pool.tile([P, s_tile], F32, name=f"ps{mi}")
                         for mi in range(mi_n)]
                pss_b = [psum_pool.tile([P, s_tile], F32, name=f"ps{mi}")
                        for mi in range(mi_n)]
                if q % 2 == 0:
                    bf_phase(pss_a, wtb_a, first=True, last=False)
                    bf_phase(pss_b, wtb_b, first=True, last=False)
                    dr_phase(pss_a, wt8_a, first=False, last=True)
                    evict(pss_a, na)
                    dr_phase(pss_b, wt8_b, first=False, last=True)
                    evict(pss_b, nb)
                else:
                    dr_phase(pss_a, wt8_a, first=True, last=False)
                    dr_phase(pss_b, wt8_b, first=True, last=False)
                    bf_phase(pss_a, wtb_a, first=False, last=True)
                    evict(pss_a, na)
                    bf_phase(pss_b, wtb_b, first=False, last=True)
                    evict(pss_b, nb)

    nc.compile()
    return nc


def _prep_in_maps(x, W_base, b_base, A, lora_B):
    # Accept jax/np arrays alike; do all host prep in numpy.
    x = np.asarray(x)
    W_base = np.asarray(W_base)
    b_base = np.asarray(b_base)
    A = np.asarray(A)
    lora_B = np.asarray(lora_B)
    # Host prep: exact fold of the LoRA path into the weight.
    W_eff = (W_base.astype(np.float32)
             + SCALING * (lora_B.astype(np.float32) @ A.astype(np.float32)))

    KF8 = N8 * P  # contraction columns handled in fp8

    # wT8[p, nO, k, o] = 8*W_eff[nO*128+o, k*128+p]  (k < N8)
    w8 = (W_eff[:, :KF8] * WS).astype(ml_dtypes.float8_e4m3)
    wT8 = np.ascontiguousarray(
        w8.reshape(NO, P, N8, P).transpose(3, 0, 2, 1)
    )
    # wTb[p, nO, k, o] = bf16(128*W_eff[nO*128+o, KF8 + k*128+p])
    wb = (W_eff[:, KF8:] * PSUM_SCALE).astype(ml_dtypes.bfloat16)
    wTb = np.ascontiguousarray(
        wb.reshape(NO, P, NBF, P).transpose(3, 0, 2, 1)
    )

    # bias[p, nO] = 128*b_base[nO*128+p]
    bias_l = np.ascontiguousarray(
        (b_base.astype(np.float32) * PSUM_SCALE).reshape(NO, P).T
    )

    xf = x.reshape(BATCH * SEQ, D_IN)
    x8_full = (xf[:, :KF8] * XS).astype(ml_dtypes.float8_e4m3)
    xb_full = xf[:, KF8:].astype(ml_dtypes.bfloat16)
    in_maps = []
    for c in range(N_CORES):
        sl = slice(c * S_PER_CORE, (c + 1) * S_PER_CORE)
        # xT8[p, mi, k, s] = e4m3(16 * x_c[mi*512+s, k*128+p])
        xT8 = np.ascontiguousarray(
            x8_full[sl].reshape(MI, S_TILE, N8, P).transpose(3, 0, 2, 1)
        )
        xTb = np.ascontiguousarray(
            xb_full[sl].reshape(MI, S_TILE, NBF, P).transpose(3, 0, 2, 1)
        )
        in_maps.append({"xT8": xT8, "xTb": xTb, "wT8": wT8, "wTb": wTb,
                        "bias": bias_l})
    return in_maps


def _unpack(res):
    out = np.empty((BATCH * SEQ, D_OUT), dtype=np.float32)
    for c in range(N_CORES):
        oc = res.results[c]["out"]  # [P, NO, MI, S_TILE]
        # out_c[mi*512+s, nO*128+p] = oc[p, nO, mi, s]
        out[c * S_PER_CORE:(c + 1) * S_PER_CORE] = (
            oc.transpose(2, 3, 1, 0).reshape(S_PER_CORE, D_OUT)
        )
    return out.reshape(BATCH, SEQ, D_OUT)


def kernel(x, W_base, b_base, A, B):
    lora_B = B
    if "nc" not in _compiled:
        _compiled["nc"] = _build_program()
    nc = _compiled["nc"]
    in_maps = _prep_in_maps(x, W_base, b_base, A, lora_B)
    res = run_bass_kernel_spmd(nc, in_maps, core_ids=list(range(N_CORES)))
    return _unpack(res)


def profiled_run(inputs, tmpdir=None, trace_cores=None):
    """Re-run the SPMD kernel with NTFF tracing; returns exec_time_ns
    (max across traced cores). Used by test.py only (requires the
    antenv.axon_hooks shim)."""
    if "nc" not in _compiled:
        _compiled["nc"] = _build_program()
    nc = _compiled["nc"]
    in_maps = _prep_in_maps(
        inputs["x"], inputs["W_base"], inputs["b_base"], inputs["A"], inputs["B"]
    )
    res = run_bass_kernel_spmd(
        nc, in_maps, core_ids=list(range(N_CORES)), trace=True, tmpdir=tmpdir,
        trace_cores=trace_cores,
    )
    print("profile tmpdir:", tmpdir)
    if res.mean_exec_time_ns is not None:
        print(f"mean exec across traced cores: {res.mean_exec_time_ns:.0f} ns; "
              f"slowest core: {res.max_exec_time_core_id}")
    return res.exec_time_ns


# revision 16
# speedup vs baseline: 1.0054x; 1.0054x over previous
"""Trainium2 Bass kernel for CascadedLoRALinear4bit.

Computes out[b,s,o] = x @ W_base^T + b_base + scaling * (x @ A^T) @ B^T
with scaling == rank/alpha == 1.0.

Strategy:
  - Algebraic fold (exact): out = x @ (W_base + B @ A)^T + b_base.
    The fold is computed on host in fp32 (0.5 GFLOP, negligible).
  - Data-parallel over tokens: the 4*4096 = 16384 tokens are sharded
    8 ways (2048 tokens per NeuronCore). W_eff^T and bias are
    replicated to all cores. No collectives needed.
  - Mixed-precision contraction split: of the 32 k-tiles (128 each),
    the first N8 are computed in fp8 e4m3 with perf_mode=DoubleRow
    (2 k-tiles per matmul, ~2x PE rate), the remaining 32-N8 in bf16.
    N8 is chosen so the worst-case relative error stays ~1.5e-2
    (fp8-only would be 3e-2; bf16-only is 1.9e-3).
  - fp8 scaling to dodge e4m3 subnormals (tiny=2^-6): x is quantized
    as e4m3(16*x), W as e4m3(8*W) -> psum accumulates 128*(x@W^T).
    The bf16 part uses bf16(x) @ bf16(128*W) so the whole PSUM is
    uniformly 128*out. Eviction computes (psum + 128*bias) * (1/128)
    in one DVE tensor_scalar op.
  - Per core: out_c^T[4096, 2048] = W_eff @ x_c^T + bias, tiled for
    the PE with fp32 PSUM accumulation; x_c^T stays fully resident in
    SBUF; W streams through as the stationary operand; each stationary
    tile is reused for 4 moving x chunks of 512 tokens.
  - Output is computed transposed (o on partitions) so the bias is a
    per-partition scalar in the DVE eviction.

Layouts (d = contraction dim on partitions everywhere):
  xT8 [128, 4, N8, 512]     e4m3(16*x), k-tiles 0..N8
  xTb [128, 4, 32-N8, 512]  bf16(x),    k-tiles N8..32
  wT8 [128, 32, N8, 128]    e4m3(8*W),  k-tiles 0..N8
  wTb [128, 32, 32-N8, 128] bf16(128*W)
  bias[128, 32]             128*b_base  (f32)
  out [128, 32, 4, 512]     out[p,nO,mi,s] = out_c[mi*512+s, nO*128+p] (f32)
"""

import sys

if "/opt/trn_rl_repo" not in sys.path:
    sys.path.insert(0, "/opt/trn_rl_repo")

import numpy as np
import ml_dtypes

import concourse.bass as bass
import concourse.mybir as mybir
import concourse.tile as tile
from concourse import bacc
from concourse.bass_utils import run_bass_kernel_spmd

# Problem dims (hardcoded per contract)
BATCH, SEQ, D_IN, D_OUT = 4, 4096, 4096, 4096
SCALING = 1.0  # rank / alpha = 16 / 16

N_CORES = 8
P = 128
S_PER_CORE = BATCH * SEQ // N_CORES  # 2048
KO = D_IN // P                       # 32 contraction tiles
S_TILE = 512
MI = S_PER_CORE // S_TILE            # 4 moving (token) chunks
NO = D_OUT // P                      # 32 output-row blocks

N8 = 10           # k-tiles computed in fp8 DoubleRow (must be even)
NBF = KO - N8     # k-tiles computed in bf16
XS = 16.0         # fp8 x pre-scale
WS = 8.0          # fp8 W pre-scale  (total PSUM scale = XS*WS = 128)
PSUM_SCALE = XS * WS

BF16 = mybir.dt.bfloat16
F8 = mybir.dt.float8e4
F32 = mybir.dt.float32

_compiled = {}


def _build_program(mi_n=MI, no_n=NO, n8=N8, nbf=NBF, s_tile=S_TILE):
    nc = bacc.Bacc(None, target_bir_lowering=False)

    xT8 = nc.declare_dram_parameter("xT8", [P, mi_n, n8, s_tile], F8, isOutput=False)
    xTb = nc.declare_dram_parameter("xTb", [P, mi_n, nbf, s_tile], BF16, isOutput=False)
    wT8 = nc.declare_dram_parameter("wT8", [P, no_n, n8, P], F8, isOutput=False)
    wTb = nc.declare_dram_parameter("wTb", [P, no_n, nbf, P], BF16, isOutput=False)
    bias_d = nc.declare_dram_parameter("bias", [P, no_n], F32, isOutput=False)
    out_d = nc.declare_dram_parameter("out", [P, no_n, mi_n, s_tile], F32, isOutput=True)

    inv_scale = 1.0 / PSUM_SCALE

    with tile.TileContext(nc) as tc:
        with (
            tc.tile_pool(name="xres", bufs=1) as x_pool,
            tc.tile_pool(name="wt", bufs=3) as wt_pool,
            tc.tile_pool(name="bias", bufs=1) as bias_pool,
            tc.tile_pool(name="o", bufs=8) as out_pool,
            tc.tile_pool(name="psum", bufs=2, space="PSUM") as psum_pool,
        ):
            # ---- Startup-latency-aware preload + a paired prologue ----
            # The DMA ring is one FIFO stream (~350 GB/s), so the 13.5 MiB
            # x preload takes ~40us while a single n-block only computes
            # for ~23us.  Blocks 0 and 1 are therefore emitted as a PAIR:
            # both fp8 DoubleRow phases first (their x is only 2.5 MiB),
            # then the two bf16 phases interleaved per k-tile, so the PE
            # consumes each arriving xb chunk twice and stays paced with
            # the stream.  DMA issue order mirrors consumption order.
            xres8 = [x_pool.tile([P, n8, s_tile], F8, name=f"x8_{mi}")
                     for mi in range(mi_n)]
            xresb = [x_pool.tile([P, nbf, s_tile], BF16, name=f"xb_{mi}")
                     for mi in range(mi_n)]

            # PE warm-up: the first real matmul can only start once its
            # DMA lands (~10us in), and the PE then crawls at the low
            # DVFS p-state for the first ~3us of activity.  Run dummy
            # matmuls on a zeroed SBUF tile during the DMA wait so the
            # clock is already ramped; they write the real PSUM tiles,
            # whose first real matmul uses start=True and so discards
            # the garbage.
            scr = x_pool.tile([P, 640], BF16, name="warmup")
            nc.vector.memset(scr[:], 0.0)

            wt80 = wt_pool.tile([P, n8, P], F8, name="wt8")
            nc.sync.dma_start(out=wt80[:], in_=wT8[:, 0, :, :])
            for mi in range(mi_n):
                nc.sync.dma_start(out=xres8[mi][:, 0:2, :],
                                  in_=xT8[:, mi, 0:2, :])
            wt81 = wt_pool.tile([P, n8, P], F8, name="wt8")
            nc.sync.dma_start(out=wt81[:], in_=wT8[:, 1, :, :])
            wtb0 = wt_pool.tile([P, nbf, P], BF16, name="wtb")
            wtb1 = wt_pool.tile([P, nbf, P], BF16, name="wtb")
            # first two bf16 k-tiles of each weight block, so the bf16
            # phases can start while the rest streams in
            nc.sync.dma_start(out=wtb0[:, 0:2, :], in_=wTb[:, 0, 0:2, :])
            nc.sync.dma_start(out=wtb1[:, 0:2, :], in_=wTb[:, 1, 0:2, :])
            for kc in range(2, n8, 2):
                for mi in range(mi_n):
                    nc.sync.dma_start(
                        out=xres8[mi][:, kc:kc + 2, :],
                        in_=xT8[:, mi, kc:kc + 2, :],
                    )
            for kc in range(0, nbf, 2):
                for mi in range(mi_n):
                    nc.sync.dma_start(
                        out=xresb[mi][:, kc:kc + 2, :],
                        in_=xTb[:, mi, kc:kc + 2, :],
                    )
                if kc == 0:
                    nc.sync.dma_start(out=wtb0[:, 2:, :], in_=wTb[:, 0, 2:, :])
                    nc.sync.dma_start(out=wtb1[:, 2:, :], in_=wTb[:, 1, 2:, :])
                elif kc == 4:
                    bias_t = bias_pool.tile([P, no_n], F32)
                    nc.sync.dma_start(out=bias_t[:], in_=bias_d[:])
                    wt82 = wt_pool.tile([P, n8, P], F8, name="wt8")
                    nc.sync.dma_start(out=wt82[:], in_=wT8[:, 2, :, :])
                    wtb2 = wt_pool.tile([P, nbf, P], BF16, name="wtb")
                    nc.sync.dma_start(out=wtb2[:], in_=wTb[:, 2, :, :])
            wt_blks = {0: (wt80, wtb0), 1: (wt81, wtb1), 2: (wt82, wtb2)}

            def dr_phase(pss, wt8_blk, first, last):
                # fp8 DoubleRow pairs: 2 k-tiles per matmul
                for j in range(0, n8, 2):
                    for mi in range(mi_n):
                        nc.tensor.matmul(
                            pss[mi][:],
                            lhsT=wt8_blk[:, j:j + 2, :],
                            rhs=xres8[mi][:, j:j + 2, :],
                            start=(first and j == 0),
                            stop=(last and j == n8 - 2),
                            perf_mode=mybir.MatmulPerfMode.DoubleRow,
                        )

            def evict(pss, n):
                for mi in range(mi_n):
                    ot = out_pool.tile([P, s_tile], F32)
                    # out = (psum + 128*bias) * (1/128)
                    nc.vector.tensor_scalar(
                        ot[:], pss[mi][:],
                        bias_t[:, n:n + 1], inv_scale,
                        mybir.AluOpType.add, mybir.AluOpType.mult,
                    )
                    nc.sync.dma_start(out=out_d[:, n, mi, :], in_=ot[:])

            # Prologue pair: blocks 0 and 1 (uses all 8 PSUM banks)
            pss0 = [psum_pool.tile([P, s_tile], F32, name=f"ps{mi}")
                    for mi in range(mi_n)]
            pss1 = [psum_pool.tile([P, s_tile], F32, name=f"ps{mi}")
                    for mi in range(mi_n)]
            for w in range(24):
                nc.tensor.matmul(
                    pss0[w % mi_n][:],
                    lhsT=scr[:, 0:P],
                    rhs=scr[:, P:P + s_tile],
                    start=True,
                    stop=True,
                )
            dr_phase(pss0, wt80, first=True, last=False)
            dr_phase(pss1, wt81, first=True, last=False)
            for k in range(nbf):
                for pss, wtb_blk in ((pss0, wtb0), (pss1, wtb1)):
                    for mi in range(mi_n):
                        nc.tensor.matmul(
                            pss[mi][:],
                            lhsT=wtb_blk[:, k, :],
                            rhs=xresb[mi][:, k, :],
                            start=False,
                            stop=(k == nbf - 1),
                        )
            evict(pss0, 0)
            evict(pss1, 1)

            def bf_phase(pss, wtb_blk, first, last):
                # bf16 k-tiles
                for k in range(nbf):
                    for mi in range(mi_n):
                        nc.tensor.matmul(
                            pss[mi][:],
                            lhsT=wtb_blk[:, k, :],
                            rhs=xresb[mi][:, k, :],
                            start=(first and k == 0),
                            stop=(last and k == nbf - 1),
                        )

            # Steady state: blocks in PAIRS with the phase pattern
            # alternating per pair ([bf,bf,DR,DR] then [DR,DR,bf,bf]),
            # so same-mode matmuls chain across every boundary: one
            # DoubleRow<->normal mode transition per pair instead of
            # two per block.  Each pair uses all 8 PSUM banks (two
            # 4-bank generations of the bufs=2 pool), and a block's
            # banks are evicted one phase before the next pair needs
            # them, so no pipeline bubble.
            for q, na in enumerate(range(2, no_n, 2)):
                nb = na + 1
                blks = []
                for n in (na, nb):
                    if n in wt_blks:
                        blks.append(wt_blks.pop(n))
                    else:
                        w8t = wt_pool.tile([P, n8, P], F8, name="wt8")
                        nc.sync.dma_start(out=w8t[:], in_=wT8[:, n, :, :])
                        wbt = wt_pool.tile([P, nbf, P], BF16, name="wtb")
                        nc.sync.dma_start(out=wbt[:], in_=wTb[:, n, :, :])
                        blks.append((w8t, wbt))
                (wt8_a, wtb_a), (wt8_b, wtb_b) = blks
                pss_a = [psum_pool.tile([P, s_tile], F32, name=f"ps{mi}")
                         for mi in range(mi_n)]
                pss_b = [psum_pool.tile([P, s_tile], F32, name=f"ps{mi}")
                        for mi in range(mi_n)]
                if q % 2 == 0:
                    bf_phase(pss_a, wtb_a, first=True, last=False)
                    bf_phase(pss_b, wtb_b, first=True, last=False)
                    dr_phase(pss_a, wt8_a, first=False, last=True)
                    evict(pss_a, na)
                    dr_phase(pss_b, wt8_b, first=False, last=True)
                    evict(pss_b, nb)
                else:
                    dr_phase(pss_a, wt8_a, first=True, last=False)
                    dr_phase(pss_b, wt8_b, first=True, last=False)
                    bf_phase(pss_a, wtb_a, first=False, last=True)
                    evict(pss_a, na)
                    bf_phase(pss_b, wtb_b, first=False, last=True)
                    evict(pss_b, nb)

    nc.compile()
    return nc


def _prep_in_maps(x, W_base, b_base, A, lora_B):
    # Accept jax/np arrays alike; do all host prep in numpy.
    x = np.asarray(x)
    W_base = np.asarray(W_base)
    b_base = np.asarray(b_base)
    A = np.asarray(A)
    lora_B = np.asarray(lora_B)
    # Host prep: exact fold of the LoRA path into the weight.
    W_eff = (W_base.astype(np.float32)
             + SCALING * (lora_B.astype(np.float32) @ A.astype(np.float32)))

    KF8 = N8 * P  # contraction columns handled in fp8

    # wT8[p, nO, k, o] = 8*W_eff[nO*128+o, k*128+p]  (k < N8)
    w8 = (W_eff[:, :KF8] * WS).astype(ml_dtypes.float8_e4m3)
    wT8 = np.ascontiguousarray(
        w8.reshape(NO, P, N8, P).transpose(3, 0, 2, 1)
    )
    # wTb[p, nO, k, o] = bf16(128*W_eff[nO*128+o, KF8 + k*128+p])
    wb = (W_eff[:, KF8:] * PSUM_SCALE).astype(ml_dtypes.bfloat16)
    wTb = np.ascontiguousarray(
        wb.reshape(NO, P, NBF, P).transpose(3, 0, 2, 1)
    )

    # bias[p, nO] = 128*b_base[nO*128+p]
    bias_l = np.ascontiguousarray(
        (b_base.astype(np.float32) * PSUM_SCALE).reshape(NO, P).T
    )

    xf = x.reshape(BATCH * SEQ, D_IN)
    x8_full = (xf[:, :KF8] * XS).astype(ml_dtypes.float8_e4m3)
    xb_full = xf[:, KF8:].astype(ml_dtypes.bfloat16)
    in_maps = []
    for c in range(N_CORES):
        sl = slice(c * S_PER_CORE, (c + 1) * S_PER_CORE)
        # xT8[p, mi, k, s] = e4m3(16 * x_c[mi*512+s, k*128+p])
        xT8 = np.ascontiguousarray(
            x8_full[sl].reshape(MI, S_TILE, N8, P).transpose(3, 0, 2, 1)
        )
        xTb = np.ascontiguousarray(
            xb_full[sl].reshape(MI, S_TILE, NBF, P).transpose(3, 0, 2, 1)
        )
        in_maps.append({"xT8": xT8, "xTb": xTb, "wT8": wT8, "wTb": wTb,
                        "bias": bias_l})
    return in_maps


def _unpack(res):
    out = np.empty((BATCH * SEQ, D_OUT), dtype=np.float32)
    for c in range(N_CORES):
        oc = res.results[c]["out"]  # [P, NO, MI, S_TILE]
        # out_c[mi*512+s, nO*128+p] = oc[p, nO, mi, s]
        out[c * S_PER_CORE:(c + 1) * S_PER_CORE] = (
            oc.transpose(2, 3, 1, 0).reshape(S_PER_CORE, D_OUT)
        )
    return out.reshape(BATCH, SEQ, D_OUT)


def kernel(x, W_base, b_base, A, B):
    lora_B = B
    if "nc" not in _compiled:
        _compiled["nc"] = _build_program()
    nc = _compiled["nc"]
    in_maps = _prep_in_maps(x, W_base, b_base, A, lora_B)
    res = run_bass_kernel_spmd(nc, in_maps, core_ids=list(range(N_CORES)))
    return _unpack(res)


def profiled_run(inputs, tmpdir=None, trace_cores=None):
    """Re-run the SPMD kernel with NTFF tracing; returns exec_time_ns
    (max across traced cores). Used by test.py only (requires the
    antenv.axon_hooks shim)."""
    if "nc" not in _compiled:
        _compiled["nc"] = _build_program()
    nc = _compiled["nc"]
    in_maps = _prep_in_maps(
        inputs["x"], inputs["W_base"], inputs["b_base"], inputs["A"], inputs["B"]
    )
    res = run_bass_kernel_spmd(
        nc, in_maps, core_ids=list(range(N_CORES)), trace=True, tmpdir=tmpdir,
        trace_cores=trace_cores,
    )
    print("profile tmpdir:", tmpdir)
    if res.mean_exec_time_ns is not None:
        print(f"mean exec across traced cores: {res.mean_exec_time_ns:.0f} ns; "
              f"slowest core: {res.max_exec_time_core_id}")
    return res.exec_time_ns


# revision 19
# speedup vs baseline: 1.0054x; 1.0000x over previous
"""Trainium2 Bass kernel for CascadedLoRALinear4bit.

Computes out[b,s,o] = x @ W_base^T + b_base + scaling * (x @ A^T) @ B^T
with scaling == rank/alpha == 1.0.

Strategy:
  - Algebraic fold (exact): out = x @ (W_base + B @ A)^T + b_base.
    The fold is computed on host in fp32 (0.5 GFLOP, negligible).
  - Data-parallel over tokens: the 4*4096 = 16384 tokens are sharded
    8 ways (2048 tokens per NeuronCore). W_eff^T and bias are
    replicated to all cores. No collectives needed.
  - Mixed-precision contraction split: of the 32 k-tiles (128 each),
    the first N8 are computed in fp8 e4m3 with perf_mode=DoubleRow
    (2 k-tiles per matmul, ~2x PE rate), the remaining 32-N8 in bf16.
    N8 is chosen so the worst-case relative error stays ~1.5e-2
    (fp8-only would be 3e-2; bf16-only is 1.9e-3).
  - fp8 scaling to dodge e4m3 subnormals (tiny=2^-6): x is quantized
    as e4m3(16*x), W as e4m3(8*W) -> psum accumulates 128*(x@W^T).
    The bf16 part uses bf16(x) @ bf16(128*W) so the whole PSUM is
    uniformly 128*out. Eviction computes (psum + 128*bias) * (1/128)
    in one DVE tensor_scalar op.
  - Per core: out_c^T[4096, 2048] = W_eff @ x_c^T + bias, tiled for
    the PE with fp32 PSUM accumulation; x_c^T stays fully resident in
    SBUF; W streams through as the stationary operand; each stationary
    tile is reused for 4 moving x chunks of 512 tokens.
  - Output is computed transposed (o on partitions) so the bias is a
    per-partition scalar in the DVE eviction.

Layouts (d = contraction dim on partitions everywhere):
  xT8 [128, 4, N8, 512]     e4m3(16*x), k-tiles 0..N8
  xTb [128, 4, 32-N8, 512]  bf16(x),    k-tiles N8..32
  wT8 [128, 32, N8, 128]    e4m3(8*W),  k-tiles 0..N8
  wTb [128, 32, 32-N8, 128] bf16(128*W)
  bias[128, 32]             128*b_base  (f32)
  out [128, 32, 4, 512]     out[p,nO,mi,s] = out_c[mi*512+s, nO*128+p] (f32)
"""

import sys

if "/opt/trn_rl_repo" not in sys.path:
    sys.path.insert(0, "/opt/trn_rl_repo")

import numpy as np
import ml_dtypes

import concourse.bass as bass
import concourse.mybir as mybir
import concourse.tile as tile
from concourse import bacc
from concourse.bass_utils import run_bass_kernel_spmd

# Problem dims (hardcoded per contract)
BATCH, SEQ, D_IN, D_OUT = 4, 4096, 4096, 4096
SCALING = 1.0  # rank / alpha = 16 / 16

N_CORES = 8
P = 128
S_PER_CORE = BATCH * SEQ // N_CORES  # 2048
KO = D_IN // P                       # 32 contraction tiles
S_TILE = 512
MI = S_PER_CORE // S_TILE            # 4 moving (token) chunks
NO = D_OUT // P                      # 32 output-row blocks

N8 = 10           # k-tiles computed in fp8 DoubleRow (must be even)
NBF = KO - N8     # k-tiles computed in bf16
XS = 16.0         # fp8 x pre-scale
WS = 8.0          # fp8 W pre-scale  (total PSUM scale = XS*WS = 128)
PSUM_SCALE = XS * WS

BF16 = mybir.dt.bfloat16
F8 = mybir.dt.float8e4
F32 = mybir.dt.float32

_compiled = {}


def _build_program(mi_n=MI, no_n=NO, n8=N8, nbf=NBF, s_tile=S_TILE):
    nc = bacc.Bacc(None, target_bir_lowering=False)

    xT8 = nc.declare_dram_parameter("xT8", [P, mi_n, n8, s_tile], F8, isOutput=False)
    xTb = nc.declare_dram_parameter("xTb", [P, mi_n, nbf, s_tile], BF16, isOutput=False)
    wT8 = nc.declare_dram_parameter("wT8", [P, no_n, n8, P], F8, isOutput=False)
    wTb = nc.declare_dram_parameter("wTb", [P, no_n, nbf, P], BF16, isOutput=False)
    bias_d = nc.declare_dram_parameter("bias", [P, no_n], F32, isOutput=False)
    out_d = nc.declare_dram_parameter("out", [P, no_n, mi_n, s_tile], F32, isOutput=True)

    inv_scale = 1.0 / PSUM_SCALE

    with tile.TileContext(nc) as tc:
        with (
            tc.tile_pool(name="xres", bufs=1) as x_pool,
            tc.tile_pool(name="wt", bufs=3) as wt_pool,
            tc.tile_pool(name="bias", bufs=1) as bias_pool,
            tc.tile_pool(name="o", bufs=8) as out_pool,
            tc.tile_pool(name="psum", bufs=2, space="PSUM") as psum_pool,
        ):
            # ---- Startup-latency-aware preload + a paired prologue ----
            # The DMA ring is one FIFO stream (~350 GB/s), so the 13.5 MiB
            # x preload takes ~40us while a single n-block only computes
            # for ~23us.  Blocks 0 and 1 are therefore emitted as a PAIR:
            # both fp8 DoubleRow phases first (their x is only 2.5 MiB),
            # then the two bf16 phases interleaved per k-tile, so the PE
            # consumes each arriving xb chunk twice and stays paced with
            # the stream.  DMA issue order mirrors consumption order.
            xres8 = [x_pool.tile([P, n8, s_tile], F8, name=f"x8_{mi}")
                     for mi in range(mi_n)]
            xresb = [x_pool.tile([P, nbf, s_tile], BF16, name=f"xb_{mi}")
                     for mi in range(mi_n)]

            # PE warm-up: the first real matmul can only start once its
            # DMA lands (~10us in), and the PE then crawls at the low
            # DVFS p-state for the first ~3us of activity.  Run dummy
            # matmuls on a zeroed SBUF tile during the DMA wait so the
            # clock is already ramped; they write the real PSUM tiles,
            # whose first real matmul uses start=True and so discards
            # the garbage.
            scr = x_pool.tile([P, 640], BF16, name="warmup")
            nc.vector.memset(scr[:], 0.0)

            # The prologue pair runs its bf16 phases FIRST (consumption
            # 3.46us per 2-k-tile chunk pair, just above the ~3us stream
            # arrival rate, so the PE stays fed), then the fp8 DoubleRow
            # phases with x8 long since resident.  DMA order mirrors
            # that: bf16 weights, then xb chunks with the small x8/fp8
            # weights slotted mid-stream, stragglers last.
            wtb0 = wt_pool.tile([P, nbf, P], BF16, name="wtb")
            nc.sync.dma_start(out=wtb0[:], in_=wTb[:, 0, :, :])
            wtb1 = wt_pool.tile([P, nbf, P], BF16, name="wtb")
            nc.sync.dma_start(out=wtb1[:], in_=wTb[:, 1, :, :])
            for kc in range(0, nbf, 2):
                for mi in range(mi_n):
                    nc.sync.dma_start(
                        out=xresb[mi][:, kc:kc + 2, :],
                        in_=xTb[:, mi, kc:kc + 2, :],
                    )
                if kc == 10:
                    for kc8 in range(0, n8, 2):
                        for mi in range(mi_n):
                            nc.sync.dma_start(
                                out=xres8[mi][:, kc8:kc8 + 2, :],
                                in_=xT8[:, mi, kc8:kc8 + 2, :],
                            )
            wt80 = wt_pool.tile([P, n8, P], F8, name="wt8")
            nc.sync.dma_start(out=wt80[:], in_=wT8[:, 0, :, :])
            wt81 = wt_pool.tile([P, n8, P], F8, name="wt8")
            nc.sync.dma_start(out=wt81[:], in_=wT8[:, 1, :, :])
            bias_t = bias_pool.tile([P, no_n], F32)
            nc.sync.dma_start(out=bias_t[:], in_=bias_d[:])
            wt82 = wt_pool.tile([P, n8, P], F8, name="wt8")
            nc.sync.dma_start(out=wt82[:], in_=wT8[:, 2, :, :])
            wtb2 = wt_pool.tile([P, nbf, P], BF16, name="wtb")
            nc.sync.dma_start(out=wtb2[:], in_=wTb[:, 2, :, :])
            wt_blks = {0: (wt80, wtb0), 1: (wt81, wtb1), 2: (wt82, wtb2)}

            def dr_phase(pss, wt8_blk, first, last):
                # fp8 DoubleRow pairs: 2 k-tiles per matmul
                for j in range(0, n8, 2):
                    for mi in range(mi_n):
                        nc.tensor.matmul(
                            pss[mi][:],
                            lhsT=wt8_blk[:, j:j + 2, :],
                            rhs=xres8[mi][:, j:j + 2, :],
                            start=(first and j == 0),
                            stop=(last and j == n8 - 2),
                            perf_mode=mybir.MatmulPerfMode.DoubleRow,
                        )

            def evict(pss, n):
                for mi in range(mi_n):
                    ot = out_pool.tile([P, s_tile], F32)
                    # out = (psum + 128*bias) * (1/128)
                    nc.vector.tensor_scalar(
                        ot[:], pss[mi][:],
                        bias_t[:, n:n + 1], inv_scale,
                        mybir.AluOpType.add, mybir.AluOpType.mult,
                    )
                    nc.sync.dma_start(out=out_d[:, n, mi, :], in_=ot[:])

            # Prologue pair: blocks 0 and 1 (uses all 8 PSUM banks)
            pss0 = [psum_pool.tile([P, s_tile], F32, name=f"ps{mi}")
                    for mi in range(mi_n)]
            pss1 = [psum_pool.tile([P, s_tile], F32, name=f"ps{mi}")
                    for mi in range(mi_n)]
            for w in range(40):
                nc.tensor.matmul(
                    pss0[w % mi_n][:],
                    lhsT=scr[:, 0:P],
                    rhs=scr[:, P:P + s_tile],
                    start=True,
                    stop=True,
                )
            for k in range(nbf):
                for pss, wtb_blk in ((pss0, wtb0), (pss1, wtb1)):
                    for mi in range(mi_n):
                        nc.tensor.matmul(
                            pss[mi][:],
                            lhsT=wtb_blk[:, k, :],
                            rhs=xresb[mi][:, k, :],
                            start=(k == 0),
                            stop=False,
                        )
            dr_phase(pss0, wt80, first=False, last=True)
            evict(pss0, 0)
            dr_phase(pss1, wt81, first=False, last=True)
            evict(pss1, 1)

            def bf_phase(pss, wtb_blk, first, last):
                # bf16 k-tiles
                for k in range(nbf):
                    for mi in range(mi_n):
                        nc.tensor.matmul(
                            pss[mi][:],
                            lhsT=wtb_blk[:, k, :],
                            rhs=xresb[mi][:, k, :],
                            start=(first and k == 0),
                            stop=(last and k == nbf - 1),
                        )

            # Steady state: blocks in PAIRS with the phase pattern
            # alternating per pair ([bf,bf,DR,DR] then [DR,DR,bf,bf]),
            # so same-mode matmuls chain across every boundary: one
            # DoubleRow<->normal mode transition per pair instead of
            # two per block.  Each pair uses all 8 PSUM banks (two
            # 4-bank generations of the bufs=2 pool), and a block's
            # banks are evicted one phase before the next pair needs
            # them, so no pipeline bubble.
            for q, na in enumerate(range(2, no_n, 2)):
                nb = na + 1
                blks = []
                for n in (na, nb):
                    if n in wt_blks:
                        blks.append(wt_blks.pop(n))
                    else:
                        w8t = wt_pool.tile([P, n8, P], F8, name="wt8")
                        nc.sync.dma_start(out=w8t[:], in_=wT8[:, n, :, :])
                        wbt = wt_pool.tile([P, nbf, P], BF16, name="wtb")
                        nc.sync.dma_start(out=wbt[:], in_=wTb[:, n, :, :])
                        blks.append((w8t, wbt))
                (wt8_a, wtb_a), (wt8_b, wtb_b) = blks
                pss_a = [psum_pool.tile([P, s_tile], F32, name=f"ps{mi}")
                         for mi in range(mi_n)]
                pss_b = [psum_pool.tile([P, s_tile], F32, name=f"ps{mi}")
                        for mi in range(mi_n)]
                if q % 2 == 0:
                    # prologue ended on DoubleRow, so chain DR first
                    dr_phase(pss_a, wt8_a, first=True, last=False)
                    dr_phase(pss_b, wt8_b, first=True, last=False)
                    bf_phase(pss_a, wtb_a, first=False, last=True)
                    evict(pss_a, na)
                    bf_phase(pss_b, wtb_b, first=False, last=True)
                    evict(pss_b, nb)
                else:
                    bf_phase(pss_a, wtb_a, first=True, last=False)
                    bf_phase(pss_b, wtb_b, first=True, last=False)
                    dr_phase(pss_a, wt8_a, first=False, last=True)
                    evict(pss_a, na)
                    dr_phase(pss_b, wt8_b, first=False, last=True)
                    evict(pss_b, nb)

    nc.compile()
    return nc


def _prep_in_maps(x, W_base, b_base, A, lora_B):
    # Accept jax/np arrays alike; do all host prep in numpy.
    x = np.asarray(x)
    W_base = np.asarray(W_base)
    b_base = np.asarray(b_base)
    A = np.asarray(A)
    lora_B = np.asarray(lora_B)
    # Host prep: exact fold of the LoRA path into the weight.
    W_eff = (W_base.astype(np.float32)
             + SCALING * (lora_B.astype(np.float32) @ A.astype(np.float32)))

    KF8 = N8 * P  # contraction columns handled in fp8

    # wT8[p, nO, k, o] = 8*W_eff[nO*128+o, k*128+p]  (k < N8)
    w8 = (W_eff[:, :KF8] * WS).astype(ml_dtypes.float8_e4m3)
    wT8 = np.ascontiguousarray(
        w8.reshape(NO, P, N8, P).transpose(3, 0, 2, 1)
    )
    # wTb[p, nO, k, o] = bf16(128*W_eff[nO*128+o, KF8 + k*128+p])
    wb = (W_eff[:, KF8:] * PSUM_SCALE).astype(ml_dtypes.bfloat16)
    wTb = np.ascontiguousarray(
        wb.reshape(NO, P, NBF, P).transpose(3, 0, 2, 1)
    )

    # bias[p, nO] = 128*b_base[nO*128+p]
    bias_l = np.ascontiguousarray(
        (b_base.astype(np.float32) * PSUM_SCALE).reshape(NO, P).T
    )

    xf = x.reshape(BATCH * SEQ, D_IN)
    x8_full = (xf[:, :KF8] * XS).astype(ml_dtypes.float8_e4m3)
    xb_full = xf[:, KF8:].astype(ml_dtypes.bfloat16)
    in_maps = []
    for c in range(N_CORES):
        sl = slice(c * S_PER_CORE, (c + 1) * S_PER_CORE)
        # xT8[p, mi, k, s] = e4m3(16 * x_c[mi*512+s, k*128+p])
        xT8 = np.ascontiguousarray(
            x8_full[sl].reshape(MI, S_TILE, N8, P).transpose(3, 0, 2, 1)
        )
        xTb = np.ascontiguousarray(
            xb_full[sl].reshape(MI, S_TILE, NBF, P).transpose(3, 0, 2, 1)
        )
        in_maps.append({"xT8": xT8, "xTb": xTb, "wT8": wT8, "wTb": wTb,
                        "bias": bias_l})
    return in_maps


def _unpack(res):
    out = np.empty((BATCH * SEQ, D_OUT), dtype=np.float32)
    for c in range(N_CORES):
        oc = res.results[c]["out"]  # [P, NO, MI, S_TILE]
        # out_c[mi*512+s, nO*128+p] = oc[p, nO, mi, s]
        out[c * S_PER_CORE:(c + 1) * S_PER_CORE] = (
            oc.transpose(2, 3, 1, 0).reshape(S_PER_CORE, D_OUT)
        )
    return out.reshape(BATCH, SEQ, D_OUT)


def kernel(x, W_base, b_base, A, B):
    lora_B = B
    if "nc" not in _compiled:
        _compiled["nc"] = _build_program()
    nc = _compiled["nc"]
    in_maps = _prep_in_maps(x, W_base, b_base, A, lora_B)
    res = run_bass_kernel_spmd(nc, in_maps, core_ids=list(range(N_CORES)))
    return _unpack(res)


def profiled_run(inputs, tmpdir=None, trace_cores=None):
    """Re-run the SPMD kernel with NTFF tracing; returns exec_time_ns
    (max across traced cores). Used by test.py only (requires the
    antenv.axon_hooks shim)."""
    if "nc" not in _compiled:
        _compiled["nc"] = _build_program()
    nc = _compiled["nc"]
    in_maps = _prep_in_maps(
        inputs["x"], inputs["W_base"], inputs["b_base"], inputs["A"], inputs["B"]
    )
    res = run_bass_kernel_spmd(
        nc, in_maps, core_ids=list(range(N_CORES)), trace=True, tmpdir=tmpdir,
        trace_cores=trace_cores,
    )
    print("profile tmpdir:", tmpdir)
    if res.mean_exec_time_ns is not None:
        print(f"mean exec across traced cores: {res.mean_exec_time_ns:.0f} ns; "
              f"slowest core: {res.max_exec_time_core_id}")
    return res.exec_time_ns


# revision 20
# speedup vs baseline: 1.0061x; 1.0006x over previous
"""Trainium2 Bass kernel for CascadedLoRALinear4bit.

Computes out[b,s,o] = x @ W_base^T + b_base + scaling * (x @ A^T) @ B^T
with scaling == rank/alpha == 1.0.

Strategy:
  - Algebraic fold (exact): out = x @ (W_base + B @ A)^T + b_base.
    The fold is computed on host in fp32 (0.5 GFLOP, negligible).
  - Data-parallel over tokens: the 4*4096 = 16384 tokens are sharded
    8 ways (2048 tokens per NeuronCore). W_eff^T and bias are
    replicated to all cores. No collectives needed.
  - Mixed-precision contraction split: of the 32 k-tiles (128 each),
    the first N8 are computed in fp8 e4m3 with perf_mode=DoubleRow
    (2 k-tiles per matmul, ~2x PE rate), the remaining 32-N8 in bf16.
    N8 is chosen so the worst-case relative error stays ~1.5e-2
    (fp8-only would be 3e-2; bf16-only is 1.9e-3).
  - fp8 scaling to dodge e4m3 subnormals (tiny=2^-6): x is quantized
    as e4m3(16*x), W as e4m3(8*W) -> psum accumulates 128*(x@W^T).
    The bf16 part uses bf16(x) @ bf16(128*W) so the whole PSUM is
    uniformly 128*out. Eviction computes (psum + 128*bias) * (1/128)
    in one DVE tensor_scalar op.
  - Per core: out_c^T[4096, 2048] = W_eff @ x_c^T + bias, tiled for
    the PE with fp32 PSUM accumulation; x_c^T stays fully resident in
    SBUF; W streams through as the stationary operand; each stationary
    tile is reused for 4 moving x chunks of 512 tokens.
  - Output is computed transposed (o on partitions) so the bias is a
    per-partition scalar in the DVE eviction.

Layouts (d = contraction dim on partitions everywhere):
  xT8 [128, 4, N8, 512]     e4m3(16*x), k-tiles 0..N8
  xTb [128, 4, 32-N8, 512]  bf16(x),    k-tiles N8..32
  wT8 [128, 32, N8, 128]    e4m3(8*W),  k-tiles 0..N8
  wTb [128, 32, 32-N8, 128] bf16(128*W)
  bias[128, 32]             128*b_base  (f32)
  out [128, 32, 4, 512]     out[p,nO,mi,s] = out_c[mi*512+s, nO*128+p] (f32)
"""

import sys

if "/opt/trn_rl_repo" not in sys.path:
    sys.path.insert(0, "/opt/trn_rl_repo")

import numpy as np
import ml_dtypes

import concourse.bass as bass
import concourse.mybir as mybir
import concourse.tile as tile
from concourse import bacc
from concourse.bass_utils import run_bass_kernel_spmd

# Problem dims (hardcoded per contract)
BATCH, SEQ, D_IN, D_OUT = 4, 4096, 4096, 4096
SCALING = 1.0  # rank / alpha = 16 / 16

N_CORES = 8
P = 128
S_PER_CORE = BATCH * SEQ // N_CORES  # 2048
KO = D_IN // P                       # 32 contraction tiles
S_TILE = 512
MI = S_PER_CORE // S_TILE            # 4 moving (token) chunks
NO = D_OUT // P                      # 32 output-row blocks

N8 = 10           # k-tiles computed in fp8 DoubleRow (must be even)
NBF = KO - N8     # k-tiles computed in bf16
XS = 16.0         # fp8 x pre-scale
WS = 8.0          # fp8 W pre-scale  (total PSUM scale = XS*WS = 128)
PSUM_SCALE = XS * WS

BF16 = mybir.dt.bfloat16
F8 = mybir.dt.float8e4
F32 = mybir.dt.float32

_compiled = {}


def _build_program(mi_n=MI, no_n=NO, n8=N8, nbf=NBF, s_tile=S_TILE):
    nc = bacc.Bacc(None, target_bir_lowering=False)

    xT8 = nc.declare_dram_parameter("xT8", [P, mi_n, n8, s_tile], F8, isOutput=False)
    xTb = nc.declare_dram_parameter("xTb", [P, mi_n, nbf, s_tile], BF16, isOutput=False)
    wT8 = nc.declare_dram_parameter("wT8", [P, no_n, n8, P], F8, isOutput=False)
    wTb = nc.declare_dram_parameter("wTb", [P, no_n, nbf, P], BF16, isOutput=False)
    bias_d = nc.declare_dram_parameter("bias", [P, no_n], F32, isOutput=False)
    out_d = nc.declare_dram_parameter("out", [P, no_n, mi_n, s_tile], F32, isOutput=True)

    inv_scale = 1.0 / PSUM_SCALE

    with tile.TileContext(nc) as tc:
        with (
            tc.tile_pool(name="xres", bufs=1) as x_pool,
            tc.tile_pool(name="wt", bufs=3) as wt_pool,
            tc.tile_pool(name="bias", bufs=1) as bias_pool,
            tc.tile_pool(name="o", bufs=8) as out_pool,
            tc.tile_pool(name="psum", bufs=2, space="PSUM") as psum_pool,
        ):
            # ---- Startup-latency-aware preload + a paired prologue ----
            # The DMA ring is one FIFO stream (~350 GB/s), so the 13.5 MiB
            # x preload takes ~40us while a single n-block only computes
            # for ~23us.  Blocks 0 and 1 are therefore emitted as a PAIR:
            # both fp8 DoubleRow phases first (their x is only 2.5 MiB),
            # then the two bf16 phases interleaved per k-tile, so the PE
            # consumes each arriving xb chunk twice and stays paced with
            # the stream.  DMA issue order mirrors consumption order.
            xres8 = [x_pool.tile([P, n8, s_tile], F8, name=f"x8_{mi}")
                     for mi in range(mi_n)]
            xresb = [x_pool.tile([P, nbf, s_tile], BF16, name=f"xb_{mi}")
                     for mi in range(mi_n)]

            # PE warm-up: the first real matmul can only start once its
            # DMA lands (~10us in), and the PE then crawls at the low
            # DVFS p-state for the first ~3us of activity.  Run dummy
            # matmuls on a zeroed SBUF tile during the DMA wait so the
            # clock is already ramped; they write the real PSUM tiles,
            # whose first real matmul uses start=True and so discards
            # the garbage.
            scr = x_pool.tile([P, 640], BF16, name="warmup")
            nc.vector.memset(scr[:], 0.0)

            # The prologue pair runs its bf16 phases FIRST (consumption
            # 3.46us per 2-k-tile chunk pair, just above the ~3us stream
            # arrival rate, so the PE stays fed), then the fp8 DoubleRow
            # phases with x8 long since resident.  DMA order mirrors
            # that: bf16 weights, then xb chunks with the small x8/fp8
            # weights slotted mid-stream, stragglers last.
            wtb0 = wt_pool.tile([P, nbf, P], BF16, name="wtb")
            nc.sync.dma_start(out=wtb0[:], in_=wTb[:, 0, :, :])
            wtb1 = wt_pool.tile([P, nbf, P], BF16, name="wtb")
            nc.sync.dma_start(out=wtb1[:], in_=wTb[:, 1, :, :])
            for kc in range(0, nbf, 2):
                for mi in range(mi_n):
                    nc.sync.dma_start(
                        out=xresb[mi][:, kc:kc + 2, :],
                        in_=xTb[:, mi, kc:kc + 2, :],
                    )
                if kc >= 12:
                    # one 0.5 MiB fp8-x slice per xb group: small enough
                    # to be absorbed by the consumption lag, done well
                    # before the DoubleRow phases need it
                    kc8 = kc - 12
                    if kc8 < n8:
                        for mi in range(mi_n):
                            nc.sync.dma_start(
                                out=xres8[mi][:, kc8:kc8 + 2, :],
                                in_=xT8[:, mi, kc8:kc8 + 2, :],
                            )
            wt80 = wt_pool.tile([P, n8, P], F8, name="wt8")
            nc.sync.dma_start(out=wt80[:], in_=wT8[:, 0, :, :])
            wt81 = wt_pool.tile([P, n8, P], F8, name="wt8")
            nc.sync.dma_start(out=wt81[:], in_=wT8[:, 1, :, :])
            bias_t = bias_pool.tile([P, no_n], F32)
            nc.sync.dma_start(out=bias_t[:], in_=bias_d[:])
            wt82 = wt_pool.tile([P, n8, P], F8, name="wt8")
            nc.sync.dma_start(out=wt82[:], in_=wT8[:, 2, :, :])
            wtb2 = wt_pool.tile([P, nbf, P], BF16, name="wtb")
            nc.sync.dma_start(out=wtb2[:], in_=wTb[:, 2, :, :])
            wt_blks = {0: (wt80, wtb0), 1: (wt81, wtb1), 2: (wt82, wtb2)}

            def dr_phase(pss, wt8_blk, first, last):
                # fp8 DoubleRow pairs: 2 k-tiles per matmul
                for j in range(0, n8, 2):
                    for mi in range(mi_n):
                        nc.tensor.matmul(
                            pss[mi][:],
                            lhsT=wt8_blk[:, j:j + 2, :],
                            rhs=xres8[mi][:, j:j + 2, :],
                            start=(first and j == 0),
                            stop=(last and j == n8 - 2),
                            perf_mode=mybir.MatmulPerfMode.DoubleRow,
                        )

            def evict(pss, n):
                for mi in range(mi_n):
                    ot = out_pool.tile([P, s_tile], F32)
                    # out = (psum + 128*bias) * (1/128)
                    nc.vector.tensor_scalar(
                        ot[:], pss[mi][:],
                        bias_t[:, n:n + 1], inv_scale,
                        mybir.AluOpType.add, mybir.AluOpType.mult,
                    )
                    nc.sync.dma_start(out=out_d[:, n, mi, :], in_=ot[:])

            # Prologue pair: blocks 0 and 1 (uses all 8 PSUM banks)
            pss0 = [psum_pool.tile([P, s_tile], F32, name=f"ps{mi}")
                    for mi in range(mi_n)]
            pss1 = [psum_pool.tile([P, s_tile], F32, name=f"ps{mi}")
                    for mi in range(mi_n)]
            for w in range(40):
                nc.tensor.matmul(
                    pss0[w % mi_n][:],
                    lhsT=scr[:, 0:P],
                    rhs=scr[:, P:P + s_tile],
                    start=True,
                    stop=True,
                )
            for k in range(nbf):
                for pss, wtb_blk in ((pss0, wtb0), (pss1, wtb1)):
                    for mi in range(mi_n):
                        nc.tensor.matmul(
                            pss[mi][:],
                            lhsT=wtb_blk[:, k, :],
                            rhs=xresb[mi][:, k, :],
                            start=(k == 0),
                            stop=False,
                        )
            dr_phase(pss0, wt80, first=False, last=True)
            evict(pss0, 0)
            dr_phase(pss1, wt81, first=False, last=True)
            evict(pss1, 1)

            def bf_phase(pss, wtb_blk, first, last):
                # bf16 k-tiles
                for k in range(nbf):
                    for mi in range(mi_n):
                        nc.tensor.matmul(
                            pss[mi][:],
                            lhsT=wtb_blk[:, k, :],
                            rhs=xresb[mi][:, k, :],
                            start=(first and k == 0),
                            stop=(last and k == nbf - 1),
                        )

            # Steady state: blocks in PAIRS with the phase pattern
            # alternating per pair ([bf,bf,DR,DR] then [DR,DR,bf,bf]),
            # so same-mode matmuls chain across every boundary: one
            # DoubleRow<->normal mode transition per pair instead of
            # two per block.  Each pair uses all 8 PSUM banks (two
            # 4-bank generations of the bufs=2 pool), and a block's
            # banks are evicted one phase before the next pair needs
            # them, so no pipeline bubble.
            for q, na in enumerate(range(2, no_n, 2)):
                nb = na + 1
                blks = []
                for n in (na, nb):
                    if n in wt_blks:
                        blks.append(wt_blks.pop(n))
                    else:
                        w8t = wt_pool.tile([P, n8, P], F8, name="wt8")
                        nc.sync.dma_start(out=w8t[:], in_=wT8[:, n, :, :])
                        wbt = wt_pool.tile([P, nbf, P], BF16, name="wtb")
                        nc.sync.dma_start(out=wbt[:], in_=wTb[:, n, :, :])
                        blks.append((w8t, wbt))
                (wt8_a, wtb_a), (wt8_b, wtb_b) = blks
                pss_a = [psum_pool.tile([P, s_tile], F32, name=f"ps{mi}")
                         for mi in range(mi_n)]
                pss_b = [psum_pool.tile([P, s_tile], F32, name=f"ps{mi}")
                        for mi in range(mi_n)]
                if q % 2 == 0:
                    # prologue ended on DoubleRow, so chain DR first
                    dr_phase(pss_a, wt8_a, first=True, last=False)
                    dr_phase(pss_b, wt8_b, first=True, last=False)
                    bf_phase(pss_a, wtb_a, first=False, last=True)
                    evict(pss_a, na)
                    bf_phase(pss_b, wtb_b, first=False, last=True)
                    evict(pss_b, nb)
                else:
                    bf_phase(pss_a, wtb_a, first=True, last=False)
                    bf_phase(pss_b, wtb_b, first=True, last=False)
                    dr_phase(pss_a, wt8_a, first=False, last=True)
                    evict(pss_a, na)
                    dr_phase(pss_b, wt8_b, first=False, last=True)
                    evict(pss_b, nb)

    nc.compile()
    return nc


def _prep_in_maps(x, W_base, b_base, A, lora_B):
    # Accept jax/np arrays alike; do all host prep in numpy.
    x = np.asarray(x)
    W_base = np.asarray(W_base)
    b_base = np.asarray(b_base)
    A = np.asarray(A)
    lora_B = np.asarray(lora_B)
    # Host prep: exact fold of the LoRA path into the weight.
    W_eff = (W_base.astype(np.float32)
             + SCALING * (lora_B.astype(np.float32) @ A.astype(np.float32)))

    KF8 = N8 * P  # contraction columns handled in fp8

    # wT8[p, nO, k, o] = 8*W_eff[nO*128+o, k*128+p]  (k < N8)
    w8 = (W_eff[:, :KF8] * WS).astype(ml_dtypes.float8_e4m3)
    wT8 = np.ascontiguousarray(
        w8.reshape(NO, P, N8, P).transpose(3, 0, 2, 1)
    )
    # wTb[p, nO, k, o] = bf16(128*W_eff[nO*128+o, KF8 + k*128+p])
    wb = (W_eff[:, KF8:] * PSUM_SCALE).astype(ml_dtypes.bfloat16)
    wTb = np.ascontiguousarray(
        wb.reshape(NO, P, NBF, P).transpose(3, 0, 2, 1)
    )

    # bias[p, nO] = 128*b_base[nO*128+p]
    bias_l = np.ascontiguousarray(
        (b_base.astype(np.float32) * PSUM_SCALE).reshape(NO, P).T
    )

    xf = x.reshape(BATCH * SEQ, D_IN)
    x8_full = (xf[:, :KF8] * XS).astype(ml_dtypes.float8_e4m3)
    xb_full = xf[:, KF8:].astype(ml_dtypes.bfloat16)
    in_maps = []
    for c in range(N_CORES):
        sl = slice(c * S_PER_CORE, (c + 1) * S_PER_CORE)
        # xT8[p, mi, k, s] = e4m3(16 * x_c[mi*512+s, k*128+p])
        xT8 = np.ascontiguousarray(
            x8_full[sl].reshape(MI, S_TILE, N8, P).transpose(3, 0, 2, 1)
        )
        xTb = np.ascontiguousarray(
            xb_full[sl].reshape(MI, S_TILE, NBF, P).transpose(3, 0, 2, 1)
        )
        in_maps.append({"xT8": xT8, "xTb": xTb, "wT8": wT8, "wTb": wTb,
                        "bias": bias_l})
    return in_maps


def _unpack(res):
    out = np.empty((BATCH * SEQ, D_OUT), dtype=np.float32)
    for c in range(N_CORES):
        oc = res.results[c]["out"]  # [P, NO, MI, S_TILE]
        # out_c[mi*512+s, nO*128+p] = oc[p, nO, mi, s]
        out[c * S_PER_CORE:(c + 1) * S_PER_CORE] = (
            oc.transpose(2, 3, 1, 0).reshape(S_PER_CORE, D_OUT)
        )
    return out.reshape(BATCH, SEQ, D_OUT)


def kernel(x, W_base, b_base, A, B):
    lora_B = B
    if "nc" not in _compiled:
        _compiled["nc"] = _build_program()
    nc = _compiled["nc"]
    in_maps = _prep_in_maps(x, W_base, b_base, A, lora_B)
    res = run_bass_kernel_spmd(nc, in_maps, core_ids=list(range(N_CORES)))
    return _unpack(res)


def profiled_run(inputs, tmpdir=None, trace_cores=None):
    """Re-run the SPMD kernel with NTFF tracing; returns exec_time_ns
    (max across traced cores). Used by test.py only (requires the
    antenv.axon_hooks shim)."""
    if "nc" not in _compiled:
        _compiled["nc"] = _build_program()
    nc = _compiled["nc"]
    in_maps = _prep_in_maps(
        inputs["x"], inputs["W_base"], inputs["b_base"], inputs["A"], inputs["B"]
    )
    res = run_bass_kernel_spmd(
        nc, in_maps, core_ids=list(range(N_CORES)), trace=True, tmpdir=tmpdir,
        trace_cores=trace_cores,
    )
    print("profile tmpdir:", tmpdir)
    if res.mean_exec_time_ns is not None:
        print(f"mean exec across traced cores: {res.mean_exec_time_ns:.0f} ns; "
              f"slowest core: {res.max_exec_time_core_id}")
    return res.exec_time_ns
